# revision 1
# baseline (speedup 1.0000x reference)
"""Trainium2 Bass kernel for nn_BlockLinear forward.

Computes y[b, o] = sum_k exp(log_weight[o, k]) * x[b, o*K + k]
for x [16384, 8192] fp32, log_weight [1024, 8] fp32.

Strategy: data-parallel over batch across 8 NeuronCores (2048 rows each).
Per core, 16 tiles of [128, 8192] stream through SBUF. The segmented
weighted reduce is computed with a single tensor_tensor_scan pass:

    state_t = (x_t + state_{t-1}) * wp_t        (DVE TensorTensorScanArith)

with host-precomputed telescoping multipliers per group of K=8:
    wp_k = w_k / w_{k+1}   for k = 0..K-3
    wp_{K-2} = w_{K-2}
    wp_{K-1} = 0            (flush: resets state at each group boundary)

so that state at offset K-2 of group g equals sum_{k<=K-2} x_k w_k, and the
last element's contribution is added separately: y = state[K-2::K] +
x[K-1::K] * w[:, K-1].  This does the multiply+grouped-reduce in ~10.4K
DVE cycles per tile instead of ~16.4K (mult + tensor_reduce), making the
kernel DMA-bound.
"""

import numpy as np

B = 16384
IN_F = 8192
OUT_F = 1024
K = 8
N_CORES = 8
P = 128

_CACHE = {}


def _build(b_shard, in_f, out_f, n_cores, x_bufs=3):
    """Build + compile the per-core Bass module (SPMD across n_cores)."""
    from concourse import bacc, tile, mybir

    k = K
    n_tiles = b_shard // P
    f32 = mybir.dt.float32
    add = mybir.AluOpType.add
    mult = mybir.AluOpType.mult

    nc = bacc.Bacc(
        "TRN2",
        target_bir_lowering=False,
        debug=False,
        enable_asserts=True,
        num_devices=n_cores,
    )
    x_d = nc.dram_tensor("x", [b_shard, in_f], f32, kind="ExternalInput")
    wp_d = nc.dram_tensor("wp", [P, in_f], f32, kind="ExternalInput")
    wl_d = nc.dram_tensor("wl", [P, out_f], f32, kind="ExternalInput")
    y_d = nc.dram_tensor("y", [b_shard, out_f], f32, kind="ExternalOutput")

    with tile.TileContext(nc) as tc:
        with (
            tc.tile_pool(name="consts", bufs=1) as cpool,
            tc.tile_pool(name="work", bufs=x_bufs) as pool,
        ):
            wp = cpool.tile([P, in_f], f32, tag="wp")
            wl = cpool.tile([P, out_f], f32, tag="wl")
            nc.sync.dma_start(out=wp[:], in_=wp_d[:])
            nc.sync.dma_start(out=wl[:], in_=wl_d[:])
            for i in range(n_tiles):
                xt = pool.tile([P, in_f], f32, tag="x")
                yt = pool.tile([P, out_f], f32, tag="y")
                nc.sync.dma_start(out=xt[:], in_=x_d[i * P : (i + 1) * P, :])
                # yt = x[:, K-1::K] * w[:, K-1]   (last element of each group)
                nc.vector.tensor_mul(yt[:], xt[:, k - 1 :: k], wl[:])
                # in-place segmented scan: state = (x + state) * wp
                nc.vector.tensor_tensor_scan(
                    xt[:], xt[:], wp[:], 0.0, op0=add, op1=mult
                )
                # yt += scan state at offset K-2 of each group
                nc.vector.tensor_add(yt[:], yt[:], xt[:, k - 2 :: k])
                nc.sync.dma_start(out=y_d[i * P : (i + 1) * P, :], in_=yt[:])
    nc.compile()
    return nc


def _prep_weights(log_weight, out_f, k):
    """Telescoping multipliers (fp64 on host, stored fp32)."""
    w = np.exp(np.asarray(log_weight, np.float64))  # [out_f, k]
    wp = np.empty_like(w)
    wp[:, : k - 2] = w[:, : k - 2] / w[:, 1 : k - 1]
    wp[:, k - 2] = w[:, k - 2]
    wp[:, k - 1] = 0.0
    wp_flat = np.ascontiguousarray(
        np.broadcast_to(wp.reshape(1, -1), (P, out_f * k)), dtype=np.float32
    )
    wl = np.ascontiguousarray(
        np.broadcast_to(w[:, k - 1].reshape(1, -1), (P, out_f)), dtype=np.float32
    )
    return wp_flat, wl


def kernel(x, log_weight):
    from concourse import bass_utils

    x = np.ascontiguousarray(np.asarray(x, dtype=np.float32))
    assert x.shape == (B, IN_F), x.shape
    b_shard = B // N_CORES

    if "nc" not in _CACHE:
        _CACHE["nc"] = _build(b_shard, IN_F, OUT_F, N_CORES)
    nc = _CACHE["nc"]

    wp_flat, wl = _prep_weights(log_weight, OUT_F, K)
    in_maps = [
        {
            "x": x[i * b_shard : (i + 1) * b_shard],
            "wp": wp_flat,
            "wl": wl,
        }
        for i in range(N_CORES)
    ]
    res = bass_utils.run_bass_kernel_spmd(nc, in_maps, core_ids=list(range(N_CORES)))
    y = np.concatenate([res.results[i]["y"] for i in range(N_CORES)], axis=0)
    return y
